# revision 89
# baseline (speedup 1.0000x reference)
# Trainium2 Bass kernel for nn_MultiHeadAttention_29154238005976 (ACAT-style conv-augmented MHA).
#
# Reference computation (B=4, L=1024, D=1024, H=16, DK=64):
#   q/k/v projections; q,k augmented by a "scrambled" depthwise-dense conv
#   (torch-style raw reshapes (b,h,l,dk)->(b, h*dk, l) scramble time/channels);
#   softmax attention per head; output projection.
#
# Sharding: 8 cores = 4 batches x 2 head-halves. All cores run an IDENTICAL
# program; per-core differences are pushed into host-side data permutations:
#   - time permutation sigma(l) = l XOR 8g applied to Q/K/V rows (g = head-half)
#   - channel permutation pi(c) = c XOR 512g applied to WQ/WK columns and to
#     conv kernel rows+columns
#   - conv "time" halves then both map to program range l2' in [0,512)
#   - boundary sig columns get a per-core 0/1 mask (true zero-padding vs
#     wrapped real data)
# Host gathers the 8 partial outputs, un-permutes rows, sums batch pairs, +bfc.
#
# Scheduling notes (cost-model driven):
#   - fp8 (e4m3) hi/lo operand splits + DoubleRow matmuls run the projections
#     and convs at 0.75x their bf16 PE cost with ~bf16 accuracy: x = hi + lo
#     where hi = fp8(x), lo = fp8(x - hi); products hi*hi + hi*lo + lo*hi
#     (lo*lo dropped, 2^-8 relative). Small-sigma weights are prescaled x32;
#     the conv's x32 is folded into T (= 32*(residual+conv)) and comes out in
#     the scores exp scale (0.125/1024).
#   - ctx uses pt as the stationary side (q on psum partitions, v moving at
#     N=65): full PE width vs the old M=65 layout, the softmax denominator
#     rides as vpa's ones column, normalization is a per-partition DVE
#     broadcast, and ctx^T for fc comes from 4 transposes per group.
#   - DMA: one shared 360GB/s engine pool served in issue order, one shared
#     HWDGE issuing ~630ns/instruction, ~2 outstanding DMAs per queue. The
#     q staging+scramble chain rides the SP queue (in priority order between
#     the K loads and cwq), the k chain rides Act, sig hi/lo splits run on
#     Act+Pool, so no queue head-of-line-blocks another chain.
#   - phase order: qproj, kproj, convq, vp 0-3 (PE filler while the sigk
#     chain completes), convk, then the software-pipelined attention
#     (scores of g+1 before ctx of g) with fc interleaved per qb-half.
#   - Output is written bf16 and upconverted on the host.
import numpy as np
import ml_dtypes

import concourse.bass as bass
import concourse.mybir as mybir
import concourse.tile as tile
from concourse import bacc
from concourse.masks import make_identity

bf16 = ml_dtypes.bfloat16
f8 = ml_dtypes.float8_e4m3
F32 = mybir.dt.float32
BF16 = mybir.dt.bfloat16
FP8 = mybir.dt.float8e4
DR = mybir.MatmulPerfMode.DoubleRow
AF = mybir.ActivationFunctionType
OP = mybir.AluOpType

B, L, DM, H, DK = 4, 1024, 1024, 16, 64
FMAX = 4
FILTER_LENGTHS = (2, 4)
N_CORES = 8
SIGW = 640
WSCALE = 32.0  # fp8 prescale for the tiny (sigma=0.02) weight tensors


def _hilo(x, scale=1.0):
    """Split x*scale into fp8 hi + lo planes stacked on a new leading axis.
    hi + lo carries ~8 effective mantissa bits (better than bf16)."""
    xs = np.asarray(x, np.float32) * scale
    hi = xs.astype(f8)
    lo = (xs - hi.astype(np.float32)).astype(f8)
    return np.ascontiguousarray(np.stack([hi, lo], axis=0))

_CACHE = {}


# ----------------------------------------------------------------------------
# program builder
# ----------------------------------------------------------------------------
def _build(flen: int, zero_bias: bool = False, upto: str = "all") -> bass.Bass:
    nc = bacc.Bacc("TRN2", target_bir_lowering=False, debug=False,
                   dynamic_dma_scratch_size=8192)

    def din(name, shape, dt):
        return nc.dram_tensor(name, list(shape), dt, kind="ExternalInput").ap()

    Qt_d = din("Qt", (2 * DM, L), FP8)
    Kt_d = din("Kt", (2 * DM, L), FP8)
    Vt_d = din("Vt", (2 * DM, L), FP8)
    Wq_d = din("WQ", (2 * DM, DM), FP8)
    Wk_d = din("WK", (2 * DM, DM), FP8)
    Wv_d = din("WV", (2 * DM, 512), FP8)
    Wfc_d = din("Wfc", (512, DM), BF16)
    cwq_d = din("cwq", (8, 128, flen * 2048), FP8)
    cwk_d = din("cwk", (8, 128, flen * 2048), FP8)
    if not zero_bias:
        bQn_d = din("bQn", (1, DM), BF16)
        bKn_d = din("bKn", (1, DM), BF16)
        bVn_d = din("bVn", (1, 512), BF16)
    em_d = din("emask", (128, 4), F32)
    out_d = nc.dram_tensor("out", [L, DM], BF16, kind="ExternalOutput").ap()

    CW_BUFS = 3 if flen == 2 else 2
    PT_BUFS = (10 if zero_bias else 10) if flen == 2 else 8
    sigw = 576 if flen == 2 else 640  # 64 + 64*nj
    nj = 8 if flen == 2 else 9

    with tile.TileContext(nc) as tc:
        sb = tc.alloc_tile_pool(name="sb", bufs=1)
        dr = tc.alloc_tile_pool(name="dr", bufs=1, space="DRAM")
        psA = tc.alloc_tile_pool(name="psA", bufs=1, space="PSUM")

        # ---- big loads, interleaved in first-use order ------------------
        # fp8 hi/lo chunk layout: a = 8*v + dm (v=0 hi plane, v=1 lo plane)
        Qt_sb = sb.tile([128, 16, L], FP8, tag="qkvt", bufs=2, name="Qt_sb")
        Wq_sb = sb.tile([128, 16, DM], FP8, name="Wq_sb")
        qtv = Qt_d.rearrange("(a p) c -> p a c", p=128)
        wqv = Wq_d.rearrange("(a p) c -> p a c", p=128)
        # X pairs full-width; W pairs split by cb column half so the cb=0
        # groups are fully fed after ~3MB instead of 4MB
        def load_xw(Xt_t, xv, W_t, wv, fine=False):
            if fine:
                # minimal fine-grained head: the very first matmuls need only
                # the leading pair's first columns
                nc.sync.dma_start(Xt_t[:, 0:2, 0:512], xv[:, 0:2, 0:512])
                nc.sync.dma_start(W_t[:, 0:2, 0:512], wv[:, 0:2, 0:512])
                nc.sync.dma_start(Xt_t[:, 8:10, 0:512], xv[:, 8:10, 0:512])
                nc.sync.dma_start(W_t[:, 8:10, 0:512], wv[:, 8:10, 0:512])
                nc.sync.dma_start(Xt_t[:, 2:4, 0:512], xv[:, 2:4, 0:512])
                nc.sync.dma_start(W_t[:, 2:4, 0:512], wv[:, 2:4, 0:512])
                nc.sync.dma_start(Xt_t[:, 10:12, 0:512], xv[:, 10:12, 0:512])
                nc.sync.dma_start(W_t[:, 10:12, 0:512], wv[:, 10:12, 0:512])
                nc.sync.dma_start(Xt_t[:, 0:4, 512:1024], xv[:, 0:4, 512:1024])
                nc.sync.dma_start(Xt_t[:, 8:12, 512:1024],
                                  xv[:, 8:12, 512:1024])
                groups = ((1,),)
            else:
                groups = ((0, 1),)
            for dmp in groups[0]:
                for v in (0, 8):
                    a = v + 4 * dmp
                    nc.sync.dma_start(Xt_t[:, a:a + 4], xv[:, a:a + 4])
                    nc.sync.dma_start(W_t[:, a:a + 4, 0:512],
                                      wv[:, a:a + 4, 0:512])
            for dmp in (0, 1):
                for v in (0, 8):
                    a = v + 4 * dmp
                    nc.sync.dma_start(W_t[:, a:a + 4, 512:1024],
                                      wv[:, a:a + 4, 512:1024])

        load_xw(Qt_sb, qtv, Wq_sb, wqv, fine=False)

        if not zero_bias:
            bQn_sb = sb.tile([1, DM], BF16, name="bQn_sb")
            nc.sync.dma_start(bQn_sb, bQn_d)
            bKn_sb = sb.tile([1, DM], BF16, name="bKn_sb")
            nc.sync.dma_start(bKn_sb, bKn_d)
            bVn_sb = sb.tile([1, 512], BF16, name="bVn_sb")
            nc.sync.dma_start(bVn_sb, bVn_d)
        else:
            bQn_sb = bKn_sb = bVn_sb = None
        em_sb = sb.tile([128, 4], F32, name="em_sb")
        nc.sync.dma_start(em_sb, em_d)

        Wk_sb = sb.tile([128, 16, DM], FP8, name="Wk_sb")
        Kt_sb = sb.tile([128, 16, L], FP8, tag="qkvt", bufs=2, name="Kt_sb")
        load_xw(Kt_sb, Kt_d.rearrange("(a p) c -> p a c", p=128),
                Wk_sb, Wk_d.rearrange("(a p) c -> p a c", p=128))

        ones1_sb = sb.tile([1, 128], BF16, name="ones1_sb")
        nc.vector.memset(ones1_sb, 1.0)
        ident_sb = sb.tile([128, 128], BF16, name="ident_sb")
        make_identity(nc, ident_sb)

        qT_sb = sb.tile([128, 4, L], BF16, name="qT_sb")
        kT_sb = sb.tile([128, 4, L], BF16, name="kT_sb")
        ctxT_sb = sb.tile([128, 4, L], BF16, name="ctxT_sb")
        # bf16 sig staging shares one buffer (q then k); fp8 hi/lo planes are
        # what the conv consumes
        sigq_sb = sb.tile([128, 8, sigw], BF16, tag="sigbf", bufs=1, name="sigq_sb")
        sigk_sb = sb.tile([128, 8, sigw], BF16, tag="sigbf", bufs=1, name="sigk_sb")
        sigq8 = [sb.tile([128, 8, sigw], FP8, name="sigq8h"),
                 sb.tile([128, 8, sigw], FP8, name="sigq8l")]
        sigk8 = [sb.tile([128, 8, sigw], FP8, name="sigk8h"),
                 sb.tile([128, 8, sigw], FP8, name="sigk8l")]
        vpa_sb = sb.tile([128, 8, 520], BF16, name="vpa_sb")
        vpa_r = vpa_sb.rearrange("p lb (hh c) -> p lb hh c", c=65)
        nc.vector.memset(vpa_r[:, :, :, 64], 1.0)  # the denominator "ones" column

        qp_d = dr.tile([L, DM], BF16, name="qp_d")
        kp_d = dr.tile([L, DM], BF16, name="kp_d")

        # ---- phase helpers ---------------------------------------------
        def proj(Xt_sb, W_sb, bn_sb, x_d, T_sb, sig_sb, sig8, pfx, seng):
            """x = X @ W + b staged to DRAM (for the sig scramble), with the
            own-half (cb=0) residual transposed on the fly into T_sb.
            dm-outer groups of 2 psums so the first matmul only needs one
            input chunk; evictions are pipelined one group behind; each cb
            half is staged in one SBUF tile, written with a single DMA, and
            its sig slabs are scrambled back in immediately."""
            xdv = x_d.rearrange("(lb p) c -> p lb c", p=128)
            sts = {}

            def evict(p):
                cb, lbs, pss = p
                st = sts[cb]
                # both lbs' transposes pack into ONE bf16 psum tile (one 2KB
                # bank), leaving 7 banks for the projection ring; one merged
                # residual copy (on DVE: Act is busy issuing the sig DMA
                # chain and GPSIMD cannot read PSUM)
                trp = None
                if cb == 0:
                    trp = psA.tile([128, 1024], BF16, tag="tr", bufs=1,
                                   name=f"tr_{pfx}_{lbs[0]}")
                for i, (lb, ps) in enumerate(zip(lbs, pss)):
                    nc.vector.tensor_scalar_mul(st[:, lb], ps, 1.0 / WSCALE)
                    if cb == 0:
                        for ct in range(4):
                            nc.tensor.matmul(
                                trp[:, i * 512 + ct * 128:i * 512 + ct * 128 + 128],
                                st[:, lb, ct * 128:ct * 128 + 128],
                                ident_sb, is_transpose=True,
                                start=(i == 0 and ct == 0),
                                stop=(i == 1 and ct == 3))
                if cb == 0:
                    # x32: T holds 32*(residual); the conv psum (sig * cw32)
                    # then accumulates at matching scale with a plain add, and
                    # the 1/1024 comes out in the scores exp scale
                    dst = T_sb[:, :, lbs[0] * 128:lbs[0] * 128 + 256]
                    nc.vector.tensor_scalar_mul(
                        dst.rearrange("p q (lb r) -> p q lb r", r=128),
                        trp.rearrange("p (lb ct r) -> p ct lb r", lb=2, r=128),
                        WSCALE)
                # staging out in lb halves: frees the st ring slot sooner (the
                # next proj's evictions reuse it) and shortens the sig chain
                if lbs[-1] == 3:
                    seng.dma_start(xdv[:, 0:4, cb * 512:cb * 512 + 512],
                                   st[:, 0:4])
                if lbs[-1] == 7:
                    seng.dma_start(xdv[:, 4:8, cb * 512:cb * 512 + 512],
                                   st[:, 4:8])
                    scramble_half(x_d, sig_sb, sig8, cb, seng)
                    if cb == 1:
                        # fp8 hi/lo splits for both halves, emitted after all
                        # staging writes so they can't block the Act queue
                        for c2 in range(2):
                            for dp2 in range(2):
                                d2 = slice(4 * c2 + 2 * dp2, 4 * c2 + 2 * dp2 + 2)
                                nc.scalar.activation(sig8[0][:, d2],
                                                     sig_sb[:, d2], AF.Copy)
                            for dp in range(2):
                                d3 = slice(4 * c2 + 2 * dp, 4 * c2 + 2 * dp + 2)
                                nc.gpsimd.tensor_tensor(sig8[1][:, d3],
                                                        sig_sb[:, d3],
                                                        sig8[0][:, d3],
                                                        OP.subtract)

            pend = None
            for cb in range(2):
                sts[cb] = sb.tile([128, 8, 512], BF16, tag="stage", bufs=2,
                                  name=f"st_{pfx}_{cb}")
                for g in range(4):
                    lbs = [2 * g, 2 * g + 1]
                    pss = []
                    for lb in lbs:
                        ps = psA.tile([128, 512], F32, tag="A", bufs=7,
                                      name=f"ps_{pfx}_{cb}_{lb}")
                        if not zero_bias:
                            nc.tensor.matmul(ps, ones1_sb[0:1, :],
                                             bn_sb[0:1, cb * 512:cb * 512 + 512],
                                             start=True, stop=False)
                        pss.append(ps)
                    # 3 hi/lo fp8 products per dm-pair via DoubleRow (lo*lo
                    # dropped: 2^-8 relative)
                    for dmp in range(4):
                        for pj, (vx, vw) in enumerate(((0, 0), (0, 8), (8, 0))):
                            for i, lb in enumerate(lbs):
                                nc.tensor.matmul(
                                    pss[i],
                                    Xt_sb[:, vx + 2 * dmp:vx + 2 * dmp + 2,
                                          lb * 128:lb * 128 + 128],
                                    W_sb[:, vw + 2 * dmp:vw + 2 * dmp + 2,
                                         cb * 512:cb * 512 + 512],
                                    perf_mode=DR,
                                    start=(zero_bias and dmp == 0 and pj == 0),
                                    stop=(dmp == 3 and pj == 2))
                    if pend is not None:
                        evict(pend)
                    pend = (cb, lbs, pss)
            evict(pend)

        def scramble_half(x_d, sig_sb, sig8, cb, eng):
            """sig[64*hib + il, dt, cols] <- qp[16*il + jh, 128*dt + 64*hib + jl]
            for the dt slabs derived from channel half cb (dt in [4cb, 4cb+4)).
            sig cols [0,64) = j in [960,1024); cols [64,640) = j in [0,576).
            After the scramble+mask, the bf16 sig half is split into fp8
            hi/lo planes (on gpsimd, which is otherwise idle here)."""
            xr = x_d.rearrange("(il jh) (dt hib jl) -> hib il dt jh jl",
                               jh=16, hib=2, jl=64)
            dts = slice(4 * cb, 4 * cb + 4)
            for hib in range(2):
                for dt in range(4 * cb, 4 * cb + 4):
                    dst = sig_sb[64 * hib:64 * hib + 64, dt, 64:64 + 64 * nj]
                    eng.dma_start(
                        dst.rearrange("p (jh jl) -> p jh jl", jl=64),
                        xr[hib, :, dt, 0:nj, :])
                eng.dma_start(sig_sb[64 * hib:64 * hib + 64, dts, 0:64],
                              xr[hib, :, dts, 15, :])
            # mask the wrap/pad boundary columns (j' = -2,-1[,512,513])
            nc.gpsimd.tensor_tensor(
                sig_sb[:, dts, 62:64], sig_sb[:, dts, 62:64],
                em_sb[:, None, 0:2].to_broadcast((128, 4, 2)), OP.mult)
            if flen == 4:
                nc.gpsimd.tensor_tensor(
                    sig_sb[:, dts, 576:578], sig_sb[:, dts, 576:578],
                    em_sb[:, None, 2:4].to_broadcast((128, 4, 2)), OP.mult)


        def load_cw(cw_d, pfx):
            """One DMA per o'-quarter of the conv weights (hi+lo planes)."""
            cwv = cw_d.rearrange("dt p (quarter pl x) -> p dt quarter pl x",
                                 quarter=4, pl=2)
            tiles = []
            for quarter in range(4):
                t = sb.tile([128, 8, 2, flen * 256], FP8, tag="cw",
                            bufs=CW_BUFS, name=f"cw_{pfx}_{quarter}")
                nc.sync.dma_start(t, cwv[:, :, quarter])
                tiles.append(t)
            return tiles

        def conv(sig8, cw_tiles, T_sb, pfx):
            """T[c,l] += conv output (x32), scrambled back into head-transposed
            tiles. conv psum tile s: partition o' = 128 s + 64 ph + k, free
            l2' = 64 hl + m."""
            Tr = T_sb.rearrange("p q (m r) -> p q m r", r=16)
            for quarter in range(4):
                pss = [psA.tile([128, 512], F32, tag="A", bufs=7,
                                name=f"cps_{pfx}_{quarter}_{si}") for si in range(2)]
                cwt = cw_tiles[quarter]
                for dtp in range(4):
                    for pj, (vs, vw) in enumerate(((0, 0), (0, 1), (1, 0))):
                        for si in range(2):
                            for f in range(flen):
                                nc.tensor.matmul(
                                    pss[si],
                                    cwt[:, 2 * dtp:2 * dtp + 2, vw,
                                        f * 256 + si * 128:f * 256 + si * 128 + 128],
                                    sig8[vs][:, 2 * dtp:2 * dtp + 2,
                                             62 + f:62 + f + 512],
                                    perf_mode=DR,
                                    start=(dtp == 0 and pj == 0 and f == 0),
                                    stop=(dtp == 3 and pj == 2 and f == flen - 1))
                for si in range(2):
                    sblk = 2 * quarter + si
                    ps = pss[si].rearrange("p (q h m) -> p q h m", q=4, h=2)
                    for ph in range(2):
                        for pe in range(2):
                            dst = Tr[64 * pe:64 * pe + 64, :, :, 2 * sblk + ph]
                            nc.vector.tensor_tensor(
                                dst, ps[64 * ph:64 * ph + 64, :, pe, :], dst,
                                OP.add)

        # ---- phase sequence --------------------------------------------
        PHASES = ["q", "k", "convq", "convk", "vp", "attn", "all"]
        lim = PHASES.index(upto)

        # q scrambles ride the idle SP queue: in program order they sit after
        # the K loads and before cwq, so SP's in-order issue gives the sig
        # chain priority over the conv-weight transfers automatically
        proj(Qt_sb, Wq_sb, bQn_sb, qp_d, qT_sb, sigq_sb, sigq8, "q", nc.sync)
        cwq_tiles = load_cw(cwq_d, "q")

        if lim >= 1:
            proj(Kt_sb, Wk_sb, bKn_sb, kp_d, kT_sb, sigk_sb, sigk8, "k",
                 nc.scalar)

        # ---- v-projection + attention + fc (software-pipelined) ---------
        odv = out_d.rearrange("(lb p) c -> p lb c", p=128)
        psB = None

        def vp_unit(lb):
            if psB is None:
                ps = psA.tile([128, 512], F32, tag="A", bufs=7,
                              name=f"psv_{lb}")
            else:
                ps = psB.tile([128, 512], F32, tag="cfc", bufs=4,
                              name=f"psv_{lb}")
            if not zero_bias:
                nc.tensor.matmul(ps, ones1_sb[0:1, :], bVn_sb[0:1, :],
                                 start=True, stop=False)
            for dmp in range(4):
                for pj, (vx, vw) in enumerate(((0, 0), (0, 8), (8, 0))):
                    nc.tensor.matmul(
                        ps,
                        Vt_sb[:, vx + 2 * dmp:vx + 2 * dmp + 2,
                              lb * 128:lb * 128 + 128],
                        Wv_sb[:, vw + 2 * dmp:vw + 2 * dmp + 2, :],
                        perf_mode=DR,
                        start=(zero_bias and dmp == 0 and pj == 0),
                        stop=(dmp == 3 and pj == 2))
            nc.vector.tensor_scalar_mul(
                vpa_r[:, lb, :, 0:64],
                ps.rearrange("p (hh c) -> p hh c", hh=8), 1.0 / WSCALE)

        def scores_half(qb, p4, half, pt_tiles):
            """QK^T + exp for two kt2 blocks of one (qb, p4) group."""
            for kt2 in (0, 1) if half == 0 else (2, 3):
                for pe in range(2):
                    ps_st = psB.tile([128, 1024], F32, tag="st", bufs=2,
                                     name=f"st_{qb}_{p4}_{kt2}_{pe}")
                    for h in range(2):
                        kt = 2 * kt2 + h
                        nc.tensor.matmul(
                            ps_st[:, 512 * h:512 * h + 512],
                            kT_sb[64 * pe:64 * pe + 64, p4, kt * 128:kt * 128 + 128],
                            qT_sb[64 * pe:64 * pe + 64, p4, qb * 512:qb * 512 + 512],
                            start=True, stop=True, tile_position=(64 * pe, 0))
                    pt = sb.tile([128, 1024], BF16, tag="pt", bufs=PT_BUFS,
                                 name=f"pt_{qb}_{p4}_{kt2}_{pe}")
                    nc.scalar.activation(pt, ps_st, AF.Exp, scale=0.125 / (WSCALE * WSCALE))
                    pt_tiles[pe][2 * kt2] = pt[:, 0:512]
                    pt_tiles[pe][2 * kt2 + 1] = pt[:, 512:1024]

        def ctx_phase(qb, p4, pt_tiles, tail=False):
            """pt is the STATIONARY side (q on psum partitions, v moving at
            N=65): full-width PE at 65 rows/kt instead of 512, and the
            denominator rides along as vpa's ones column -> per-partition
            reciprocal broadcast on DVE (no broadcast matmuls)."""
            cstage = sb.tile([128, 4, 128], BF16, tag="cstage", bufs=2,
                             name=f"cst_{qb}_{p4}")
            for pe in range(2):
                hl = 2 * p4 + pe
                ps_cv = psB.tile([128, 4, 65], F32, tag="cfc", bufs=4,
                                 name=f"ctx_{qb}_{p4}_{pe}")
                # kt-outer: the newest exp's pt block is only needed by
                # the last 4 matmuls instead of matmul 8 of 32
                for kt in range(8):
                    for qt in range(4):
                        nc.tensor.matmul(
                            ps_cv[:, qt, :],
                            pt_tiles[pe][kt][:, qt * 128:qt * 128 + 128],
                            vpa_sb[:, kt, 65 * hl:65 * hl + 65],
                            start=(kt == 0 and qt == 0),
                            stop=(kt == 7 and qt == 3))
                rec = sb.tile([128, 4], BF16, tag="recipb", bufs=2,
                              name=f"rcb_{qb}_{p4}_{pe}")
                with nc.allow_low_precision(reason="softmax denominators are "
                                            "O(100); bf16 reciprocal is ample"):
                    nc.vector.reciprocal(rec, ps_cv[:, :, 64])
                nc.vector.tensor_tensor(
                    cstage[:, :, 64 * pe:64 * pe + 64], ps_cv[:, :, 0:64],
                    rec[:, :, None].to_broadcast((128, 4, 64)), OP.mult)
            return (qb, p4, cstage)

        def ctx_finish(state):
            qb, p4, cstage = state
            trp = psB.tile([128, 4, 128], BF16, tag="cfc", bufs=4,
                           name=f"ctr_{qb}_{p4}")
            for qt in range(4):
                nc.tensor.matmul(trp[:, qt, :], cstage[:, qt, :], ident_sb,
                                 is_transpose=True, start=(qt == 0),
                                 stop=(qt == 3))
            nc.vector.tensor_copy(
                ctxT_sb[:, p4, qb * 512:qb * 512 + 512],
                trp.rearrange("p qt r -> p (qt r)"))

        def fc_unit(lb, last=False, use_st=False):
            # evictions split across Act (db0) and DVE (db1) so the final
            # units drain in parallel; out DMAs on Pool (SWDGE) except the
            # very last, which goes through the idle SP queue (HWDGE).
            # use_st: trailing units borrow the idle scores psum ring for db1
            # so they don't wait on cfc slots held by the final divide chains
            ost = sb.tile([128, 2, 512], BF16, tag="ostage", bufs=3,
                          name=f"ost_{lb}")
            for db in range(2):
                if use_st:
                    ps = psB.tile([128, 1024], F32, tag="st", bufs=2,
                                  name=f"fcs_{lb}_{db}")[:, 0:512]
                else:
                    ps = psB.tile([128, 512], F32, tag="cfc", bufs=4,
                                  name=f"fc_{lb}_{db}")
                for t4 in range(4):
                    nc.tensor.matmul(
                        ps, ctxT_sb[:, t4, lb * 128:lb * 128 + 128],
                        Wfc_sb[:, t4, db * 512:db * 512 + 512],
                        start=(t4 == 0), stop=(t4 == 3))
                if db == 0 and last:
                    nc.scalar.activation(ost[:, db], ps, AF.Copy)
                else:
                    nc.vector.tensor_copy(ost[:, db], ps)
                if last:
                    eng = nc.gpsimd if db == 0 else nc.sync
                    eng.dma_start(odv[:, lb, db * 512:db * 512 + 512],
                                  ost[:, db])
            if not last:
                nc.gpsimd.dma_start(odv[:, lb, :],
                                    ost.rearrange("p db c -> p (db c)"))

        # SP order continues: Vt, Wv, then cwk (ring-paced by convq)
        Vt_sb = sb.tile([128, 16, L], FP8, tag="qkvt", bufs=2, name="Vt_sb")
        vtv = Vt_d.rearrange("(a p) c -> p a c", p=128)
        for v in (0, 8):
            nc.sync.dma_start(Vt_sb[:, v:v + 4], vtv[:, v:v + 4])
            nc.sync.dma_start(Vt_sb[:, v + 4:v + 8], vtv[:, v + 4:v + 8])
        Wv_sb = sb.tile([128, 16, 512], FP8, name="Wv_sb")
        nc.sync.dma_start(Wv_sb, Wv_d.rearrange("(a p) c -> p a c", p=128))
        cwk_tiles = load_cw(cwk_d, "k")
        Wfc_sb = sb.tile([128, 4, DM], BF16, name="Wfc_sb")
        wfv = Wfc_d.rearrange("(t p) c -> p t c", p=128)
        nc.gpsimd.dma_start(Wfc_sb[:, 0:2], wfv[:, 0:2])
        nc.gpsimd.dma_start(Wfc_sb[:, 2:4], wfv[:, 2:4])
        # conv(q) -> vp 0-2 (PE filler while the sigk chain completes) ->
        # conv(k) -> vp3: the psA->psB pool boundary then waits only on vp3's
        # single eviction instead of convk's whole eviction chain
        if lim >= 2:
            conv(sigq8, cwq_tiles, qT_sb, "q")
        if lim >= 4:
            for lb in range(4 if lim >= 5 else 8):
                vp_unit(lb)
        if lim >= 3:
            conv(sigk8, cwk_tiles, kT_sb, "k")

        psA.release()
        psB = tc.alloc_tile_pool(name="psB", bufs=1, space="PSUM")

        if lim >= 5:
            # the remaining vp units weave into the first two score groups to
            # cover the st-psum ring warmup; afterwards the steady pattern is
            # Sa(g) | C(g-1) | Sb(g) | fc-unit
            groups = [(qb, p4) for qb in range(2) for p4 in range(4)]
            pt0 = [[None] * 8 for _ in range(2)]
            pt1 = [[None] * 8 for _ in range(2)]
            vp_unit(4)
            scores_half(0, 0, 0, pt0)
            vp_unit(5)
            scores_half(0, 0, 1, pt0)
            vp_unit(6)
            scores_half(0, 1, 0, pt1)
            vp_unit(7)
            cfin = ctx_phase(0, 0, pt0)
            scores_half(0, 1, 1, pt1)
            pend = (0, 1, pt1)
            fc_ready = []  # lb units whose qb-half of ctxT is complete
            for qb, p4 in groups[2:]:
                pt_tiles = [[None] * 8 for _ in range(2)]
                scores_half(qb, p4, 0, pt_tiles)
                if cfin is not None:
                    ctx_finish(cfin)
                    cfin = None
                cfin = ctx_phase(*pend)
                if pend[1] == 3:
                    # the qb-half's fc units read all four p4 column groups:
                    # finish this group NOW so the writes exist before any
                    # fc_unit emission (deferred finish would be a race)
                    ctx_finish(cfin)
                    cfin = None
                    if lim >= 6:
                        fc_ready.extend(range(4 * pend[0], 4 * pend[0] + 4))
                scores_half(qb, p4, 1, pt_tiles)
                if lim >= 6 and fc_ready:
                    fc_unit(fc_ready.pop(0))
                pend = (qb, p4, pt_tiles)
            if lim >= 6 and fc_ready:
                fc_unit(fc_ready.pop(0))
            if cfin is not None:
                ctx_finish(cfin)
            cfin = ctx_phase(*pend, tail=True)
            ctx_finish(cfin)
            if lim >= 6:
                if fc_ready:
                    fc_unit(fc_ready.pop(0))
                fc_ready.extend(range(4, 8))
                for lb in fc_ready:
                    fc_unit(lb, last=(lb == 7))

        psB.release()
        sb.release()
        dr.release()

    nc.finalize()
    return nc


# ----------------------------------------------------------------------------
# host-side data prep
# ----------------------------------------------------------------------------
def _host_prep(inp, flen, zero_bias):
    """Build the 8 per-core input dicts (core ci = 2*b + g)."""
    # per-parity shared tensors (g = 0, 1)
    shared = []
    for g in range(2):
        pi = np.arange(DM) ^ (512 * g)
        d = {}
        d["WQ"] = _hilo(inp["WQ"][:, pi], WSCALE).reshape(2 * DM, DM)
        d["WK"] = _hilo(inp["WK"][:, pi], WSCALE).reshape(2 * DM, DM)
        d["WV"] = _hilo(inp["WV"][:, 512 * g:512 * g + 512], WSCALE).reshape(2 * DM, 512)
        d["Wfc"] = np.ascontiguousarray(inp["Wfc"][512 * g:512 * g + 512, :]).astype(bf16)
        if not zero_bias:
            bQ = inp["bQ"][pi].astype(np.float32) * WSCALE
            bK = inp["bK"][pi].astype(np.float32) * WSCALE
            bV = inp["bV"][512 * g:512 * g + 512].astype(np.float32) * WSCALE
            d["bQn"] = bQ[None, :].astype(bf16)
            d["bKn"] = bK[None, :].astype(bf16)
            d["bVn"] = bV[None, :].astype(bf16)
        for name, key in (("cwq", "conv_q"), ("cwk", "conv_k")):
            c = np.asarray(inp[key])[:, :, :flen].astype(np.float32)  # (d, o, f)
            c = np.ascontiguousarray(c.transpose(2, 0, 1))            # (f, d, o)
            c = c[:, pi, :][:, :, pi]
            # layout (8 dt, 128 p, 4 quarter, 2 plane, flen f, 256): column
            # grouping so each conv pass loads its own o'-quarter, both fp8
            # hi/lo planes of the x32-scaled weights in one DMA
            cs = c.transpose(1, 0, 2).reshape(8, 128, flen, 4, 256) * WSCALE
            hi = cs.astype(f8)
            lo = (cs - hi.astype(np.float32)).astype(f8)
            hl = np.stack([hi, lo], axis=0)          # (2, dt, p, f, quarter, o)
            hl = hl.transpose(1, 2, 4, 0, 3, 5)      # (dt, p, quarter, pl, f, o)
            d[name] = np.ascontiguousarray(hl).reshape(8, 128, flen * 2048)
        em = np.zeros((128, 4), np.float32)
        em[:, :] = np.array([0, 0, 1, 1], np.float32) if g == 0 else \
            np.array([1, 1, 0, 0], np.float32)
        d["emask"] = em
        shared.append(d)

    maps = []
    for b in range(B):
        for g in range(2):
            sigma = np.arange(L) ^ (8 * g)
            m = dict(shared[g])
            m["Qt"] = _hilo(np.asarray(inp["Q"])[b][sigma, :].T).reshape(2 * DM, L)
            m["Kt"] = _hilo(np.asarray(inp["K"])[b][sigma, :].T).reshape(2 * DM, L)
            m["Vt"] = _hilo(np.asarray(inp["V"])[b][sigma, :].T).reshape(2 * DM, L)
            maps.append(m)
    return maps


def _combine(results, inp):
    out = np.zeros((B, L, DM), np.float32)
    for b in range(B):
        for g in range(2):
            sigma = np.arange(L) ^ (8 * g)
            out[b] += np.asarray(results[2 * b + g]["out"]).astype(np.float32)[sigma, :]
        out[b] += np.asarray(inp["bfc"], dtype=np.float32)
    return out


def _get_program(flen, zero_bias=False):
    key = (flen, zero_bias)
    if key not in _CACHE:
        _CACHE[key] = _build(flen, zero_bias=zero_bias)
    return _CACHE[key]


def run_on_cores(inputs, trace=False):
    """Run the SPMD kernel; returns (full_output, BassKernelResults)."""
    from concourse.bass_utils import run_bass_kernel_spmd
    inp = {k: np.asarray(v) for k, v in inputs.items()}
    f_s = np.array(FILTER_LENGTHS, np.float32)
    flen = int(FILTER_LENGTHS[int(np.argmax(f_s * np.asarray(inp["w"], np.float32)))])
    zb = all(not np.any(np.asarray(inp[k])) for k in ("bQ", "bK", "bV"))
    nc = _get_program(flen, zero_bias=zb)
    in_maps = _host_prep(inp, flen, zb)
    res = run_bass_kernel_spmd(nc, in_maps, list(range(N_CORES)), trace=trace)
    return _combine(res.results, inp), res


def kernel(**inputs) -> np.ndarray:
    out, _ = run_on_cores(inputs, trace=False)
    return out

